# revision 23
# baseline (speedup 1.0000x reference)
"""Trainium2 Bass kernel for a convolutional GRU (nn_ConvolutionalRNN).

Reference semantics (per timestep t, torch-GRUCell-style with conv1d gates):
    gi = conv1d(x[t], w_ih) + b_ih          # [B, 3C, L], precomputable
    gh = conv1d(h,    w_hh) + b_hh          # [B, 3C, L], recurrent
    r = sigmoid(gi_r + gh_r); z = sigmoid(gi_z + gh_z)
    n = tanh(gi_n + r * gh_n)
    h = n + z * (h - n)  =  z*h + n*(1-z)
    ys[t] = h

Sharding: data-parallel over batch: B=16 across 8 cores -> BL=2 rows/core.
The two batch rows per core are INDEPENDENT recurrences; they are run as
two skewed software pipelines (chains) so engine work of row 1 fills the
dependency stalls of row 0.

v2 design (vs fp32r baseline at ~950us):
 - fp16 everywhere on chip (PE fp16 = 1 cyc/row always; DVE 16-bit 2x
   modes; x and ys live in HBM as fp16 - host converts, halving DMA).
 - r and z computed in ONE sigmoid over 128 partitions (bias port takes
   the per-partition brz vector).
 - biases folded into free ports: brz -> sigmoid bias, bihn -> tanh bias,
   bhhn -> scalar_tensor_tensor scalar operand. No bias adds anywhere.
 - input-side conv taps 0,1 K-packed to 128 partitions (x is DMA'd twice,
   the second copy shifted by one column) -> 2 MMs instead of 3.
 - per-(t,b) PSUM banks: bankA = pre_rz [128,256], bankB = [gh_n; i_n]
   [128,256] so the skewed chains never share a PSUM bank.
 - z*h runs on GPSIMD (idle engine), zc=1-z on the DVE in the tanh wait
   slot; the critical DVE chain is stt(t1) -> add(t2) -> mul -> add.
 - 32 dummy matmuls at kernel start warm the PE HAM clock gate to 2.4GHz
   (the baseline ran 100% of matmuls at the cold 1.2GHz rate).
"""

import numpy as np
from contextlib import ExitStack

from concourse import bacc, mybir
import concourse.tile as tile
from concourse.bass_utils import run_bass_kernel_spmd

T, B, CIN, COUT, L = 128, 16, 64, 64, 256
GATES = 3 * COUT
NCORES = 8
BL = B // NCORES          # batch rows per core = 2
LP = L + 2                # padded length (zero border at col 0 and L+1)
NB = 6                    # x buffer depth (steps of DMA lookahead)
F32 = mybir.dt.float32
F16 = mybir.dt.float16
AF = mybir.ActivationFunctionType
ALU = mybir.AluOpType


def _build_nc():
    nc = bacc.Bacc(trn_type="TRN2", target_bir_lowering=False, debug=False)

    # Per-core DRAM I/O (fp16 data; fp32 biases).
    x_d = nc.dram_tensor("x", [CIN, T, BL, L], F16, kind="ExternalInput").ap()
    h0_d = nc.dram_tensor("h0", [COUT, BL, L], F16, kind="ExternalInput").ap()
    wprz_d = nc.dram_tensor("wprz", [2 * CIN, 2 * COUT], F16, kind="ExternalInput").ap()
    w2rz_d = nc.dram_tensor("w2rz", [CIN, 2 * COUT], F16, kind="ExternalInput").ap()
    wpn_d = nc.dram_tensor("wpn", [2 * CIN, COUT], F16, kind="ExternalInput").ap()
    w2n_d = nc.dram_tensor("w2n", [CIN, COUT], F16, kind="ExternalInput").ap()
    whhprz_d = nc.dram_tensor("whhprz", [2 * COUT, 2 * COUT], F16, kind="ExternalInput").ap()
    whh1rz_d = nc.dram_tensor("whh1rz", [COUT, 2 * COUT], F16, kind="ExternalInput").ap()
    whhpn_d = nc.dram_tensor("whhpn", [2 * COUT, COUT], F16, kind="ExternalInput").ap()
    whh1n_d = nc.dram_tensor("whh1n", [COUT, COUT], F16, kind="ExternalInput").ap()
    brz_d = nc.dram_tensor("brz", [2 * COUT, 1], F32, kind="ExternalInput").ap()
    bihn_d = nc.dram_tensor("bihn", [COUT, 1], F32, kind="ExternalInput").ap()
    bhhn_d = nc.dram_tensor("bhhn", [COUT, 1], F32, kind="ExternalInput").ap()
    ys_d = nc.dram_tensor("ys", [T, COUT, BL, L], F16, kind="ExternalOutput").ap()

    with tile.TileContext(nc) as tc, ExitStack() as ctx:
        persist = ctx.enter_context(tc.tile_pool(name="persist", bufs=1))
        work = ctx.enter_context(tc.tile_pool(name="work", bufs=2))
        psA = ctx.enter_context(tc.tile_pool(name="psA", bufs=4, space="PSUM"))
        psB = ctx.enter_context(tc.tile_pool(name="psB", bufs=4, space="PSUM"))

        # --- one-time setup -------------------------------------------------
        # Input-side weights, taps 0+1 K-packed (rows 0-63 tap0, 64-127 tap1).
        wprz = persist.tile([2 * CIN, 2 * COUT], F16)
        w2rz = persist.tile([CIN, 2 * COUT], F16)
        wpn = persist.tile([2 * CIN, COUT], F16)
        w2n = persist.tile([CIN, COUT], F16)
        # Recurrent weights, tap-packed like the input side: the h tile keeps
        # h on partitions 64-127 (cols = l+1) and a left-shifted copy on
        # partitions 0-63 (cols = l+1 shifted), so taps (0, 2) run as one
        # K=128 matmul and tap 1 as a K=64 matmul on partitions 64-127.
        whhprz = persist.tile([2 * COUT, 2 * COUT], F16)   # [tap2; tap0]
        whhpn = persist.tile([2 * COUT, COUT], F16)
        whh1rz = persist.tile([2 * COUT, 2 * COUT], F16, name="whh1rz")
        whh1n = persist.tile([2 * COUT, COUT], F16, name="whh1n")
        for t_, d_ in ((wprz, wprz_d), (w2rz, w2rz_d), (wpn, wpn_d),
                       (w2n, w2n_d), (whhprz, whhprz_d), (whhpn, whhpn_d)):
            nc.sync.dma_start(out=t_[:], in_=d_)
        nc.sync.dma_start(out=whh1rz[COUT:2 * COUT], in_=whh1rz_d)
        nc.sync.dma_start(out=whh1n[COUT:2 * COUT], in_=whh1n_d)

        brz = persist.tile([2 * COUT, 1], F32)
        bihn = persist.tile([COUT, 1], F32)
        bhhn = persist.tile([COUT, 1], F32)
        nc.sync.dma_start(out=brz[:], in_=brz_d)
        nc.sync.dma_start(out=bihn[:], in_=bihn_d)
        nc.sync.dma_start(out=bhhn[:], in_=bhhn_d)

        # h state, one tile per batch row, on partitions 64-127; interior
        # cols 1..L, zero halo at cols 0 and L+1.
        hb = [persist.tile([2 * COUT, LP], F16, tag=f"h{b}", name=f"h{b}")
              for b in range(BL)]
        for b in range(BL):
            nc.vector.memset(hb[b][:], 0.0)
            nc.sync.dma_start(out=hb[b][COUT:2 * COUT, 1:L + 1], in_=h0_d[:, b, :])
            nc.sync.dma_start(out=hb[b][0:COUT, 0:L - 1], in_=h0_d[:, b, 1:L])

        # x buffers: rows 0-63 = x shifted right by 1 col (tap0 view),
        # rows 64-127 = x (tap1 view). Col 0 of rows 0-63 must stay zero.
        xbuf = persist.tile([2 * CIN, NB, BL, LP], F16)
        nc.vector.memset(xbuf[:], 0.0)

        def dma_x(t):
            """DMA x for steps t and t+1 in one pair of descriptors (the
            sync-queue DMA-issue instruction costs ~575ns, so batch)."""
            nsteps = min(2, T - t)
            i = t % NB
            xb = xbuf[:, i:i + nsteps]
            nc.sync.dma_start(out=xb[0:CIN, :, :, 1:L + 1],
                              in_=x_d[:, t:t + nsteps])
            nc.sync.dma_start(out=xb[CIN:2 * CIN, :, :, 0:L],
                              in_=x_d[:, t:t + nsteps])

        # --- PE warmup: ~16 dense N=512 matmuls flip the HAM gate ----------
        warm = persist.tile([128, 512], F16)
        nc.vector.memset(warm[:], 0.0)
        wps = psA.tile([128, BL, L], F32, tag="bankA")
        for _ in range(16):
            nc.tensor.matmul(wps[:], warm[:, 0:128], warm[:, 0:2 * L],
                             start=True, stop=True)

        for t in range(0, min(NB - 2, T), 2):
            dma_x(t)

        def wih_mms(t):
            """Input-side convs for step t, both batch rows: 4 MMs at N=512.

            The two skewed chains share these banks; Tile's PSUM bank-hazard
            tracking serializes row 1's recurrent MMs behind row 0's reads,
            which is exactly the skew we want anyway."""
            xb = xbuf[:, t % NB]
            bankA = psA.tile([128, BL, L], F32, tag="bankA")
            bankB = psB.tile([128, BL, L], F32, tag="bankB")
            nc.tensor.matmul(bankA[:], wprz[:], xb[:, :, 0:L],
                             start=True, stop=False)
            nc.tensor.matmul(bankA[:], w2rz[:], xb[0:CIN, :, 2:LP],
                             start=False, stop=False)
            nc.tensor.matmul(bankB[COUT:2 * COUT], wpn[:], xb[:, :, 0:L],
                             start=True, stop=False)
            nc.tensor.matmul(bankB[COUT:2 * COUT], w2n[:], xb[0:CIN, :, 2:LP],
                             start=False, stop=True)
            return bankA, bankB

        def whh_mms(t, b, bankAB):
            """Recurrent convs for (t, b): 4 MMs at N=256, need h(t-1).

            h tile layout: rows 64-127 col c = h[c-1]; rows 0-63 col c =
            h[c+1]. Packed MM at col offset 0 contributes tap0 (h[l-1], rows
            64-127) + tap2 (h[l+1], rows 0-63); the K=64 MM at offset 1
            contributes tap1 (h[l])."""
            bankA, bankB = bankAB
            h = hb[b]
            nc.tensor.matmul(bankA[:, b], whhprz[:], h[:, 0:L],
                             start=False, stop=False)
            nc.tensor.matmul(bankA[:, b], whh1rz[COUT:2 * COUT],
                             h[COUT:2 * COUT, 1:L + 1],
                             start=False, stop=True)
            nc.tensor.matmul(bankB[0:COUT, b], whhpn[:], h[:, 0:L],
                             start=True, stop=False)
            nc.tensor.matmul(bankB[0:COUT, b], whh1n[COUT:2 * COUT],
                             h[COUT:2 * COUT, 1:L + 1],
                             start=False, stop=True)

        # Prime: input convs for step 0.
        banks = [None] * T
        banks[0] = wih_mms(0)

        for t in range(T):
            if t % 2 == 0 and t + NB - 2 < T:
                dma_x(t + NB - 2)
            for b in range(BL):
                bankA, bankB = banks[t]
                whh_mms(t, b, banks[t])
                # Prefetch next step's input convs while this chain's
                # elementwise tail runs (keeps the PE dense and warm).
                if b == 0 and t + 1 < T:
                    banks[t + 1] = wih_mms(t + 1)

                # The whole elementwise tail lives on partitions 64-127 so
                # every SBUF+SBUF operand pair shares a base partition (BIR
                # verifier requirement) and gpsimd work lands on cores 4-7.
                h = hb[b]
                rz = work.tile([2 * COUT, L], F16, tag=f"rz{b}")
                nc.scalar.activation(rz[:], bankA[:, b], AF.Sigmoid,
                                     bias=brz[:])
                # t1 = (gh_n + bhhn) * r ; t2 = t1 + i_n ; n = tanh(t2 + bihn)
                t1 = work.tile([2 * COUT, L], F16, tag=f"t1{b}")
                nc.vector.scalar_tensor_tensor(t1[COUT:2 * COUT],
                                               bankB[0:COUT, b], bhhn[:],
                                               rz[0:COUT], op0=ALU.add,
                                               op1=ALU.mult)
                t2 = work.tile([2 * COUT, L], F16, tag=f"t2{b}")
                nc.vector.tensor_add(t2[COUT:2 * COUT], t1[COUT:2 * COUT],
                                     bankB[COUT:2 * COUT, b])
                n = work.tile([2 * COUT, L], F16, tag=f"n{b}")
                nc.scalar.activation(n[COUT:2 * COUT], t2[COUT:2 * COUT],
                                     AF.Tanh, bias=bihn[:])
                # zc = 1 - z and zh = z*h both on GPSIMD (cores 4-7; all
                # operands on partitions 64-127), keeping the DVE free for
                # the critical chain.
                zc = work.tile([2 * COUT, L], F16, tag=f"zc{b}")
                nc.gpsimd.tensor_scalar(zc[COUT:2 * COUT],
                                        rz[COUT:2 * COUT], -1.0, 1.0,
                                        op0=ALU.mult, op1=ALU.add)
                zh = work.tile([2 * COUT, L], F16, tag=f"zh{b}")
                nc.gpsimd.tensor_mul(zh[COUT:2 * COUT], rz[COUT:2 * COUT],
                                     h[COUT:2 * COUT, 1:L + 1])
                # h_new = z*h + n*zc, then refresh the shifted copy on
                # partitions 0-63 (tensor_copy runs in the DVE 4x mode).
                nzc = work.tile([2 * COUT, L], F16, tag=f"nzc{b}")
                nc.vector.tensor_mul(nzc[COUT:2 * COUT], n[COUT:2 * COUT],
                                     zc[COUT:2 * COUT])
                nc.vector.tensor_add(h[COUT:2 * COUT, 1:L + 1],
                                     zh[COUT:2 * COUT], nzc[COUT:2 * COUT])
                nc.vector.tensor_copy(h[0:COUT, 0:L - 1],
                                      h[COUT:2 * COUT, 2:L + 1])
                # ys DMA issued from the gpsimd queue: it waits on h(t), and
                # on the sync queue that wait would block the x-in DMAs
                # queued behind it (FIFO).
                nc.gpsimd.dma_start(out=ys_d[t, :, b, :],
                                    in_=h[COUT:2 * COUT, 1:L + 1])

    nc.compile()
    return nc


_NC = None


def _get_nc():
    global _NC
    if _NC is None:
        _NC = _build_nc()
    return _NC


def _prep_in_maps(x, h0, w_ih, w_hh, b_ih, b_hh):
    w_ih = np.asarray(w_ih, np.float32)   # [GATES, CIN, 3]
    w_hh = np.asarray(w_hh, np.float32)
    b_ih = np.asarray(b_ih, np.float32)
    b_hh = np.asarray(b_hh, np.float32)

    # lhsT layouts: [K, M] with K = input-channel rows, M = gate cols.
    def packed(w, g0, g1):   # taps 0,1 stacked on K
        return np.concatenate(
            [np.transpose(w[g0:g1, :, 0], (1, 0)),
             np.transpose(w[g0:g1, :, 1], (1, 0))], axis=0).astype(np.float16)

    def tap2(w, g0, g1):
        return np.ascontiguousarray(
            np.transpose(w[g0:g1, :, 2], (1, 0))).astype(np.float16)

    wprz = packed(w_ih, 0, 2 * COUT)
    wpn = packed(w_ih, 2 * COUT, GATES)
    w2rz = tap2(w_ih, 0, 2 * COUT)
    w2n = tap2(w_ih, 2 * COUT, GATES)

    # Recurrent weights: [tap2; tap0] K-packed + tap1 single.
    def hpacked(w, g0, g1):
        return np.concatenate(
            [np.transpose(w[g0:g1, :, 2], (1, 0)),
             np.transpose(w[g0:g1, :, 0], (1, 0))], axis=0).astype(np.float16)

    def htap1(w, g0, g1):
        return np.ascontiguousarray(
            np.transpose(w[g0:g1, :, 1], (1, 0))).astype(np.float16)

    whhprz = hpacked(w_hh, 0, 2 * COUT)
    whh1rz = htap1(w_hh, 0, 2 * COUT)
    whhpn = hpacked(w_hh, 2 * COUT, GATES)
    whh1n = htap1(w_hh, 2 * COUT, GATES)

    brz = (b_ih[:2 * COUT] + b_hh[:2 * COUT]).reshape(2 * COUT, 1)
    bihn = b_ih[2 * COUT:].reshape(COUT, 1)
    bhhn = b_hh[2 * COUT:].reshape(COUT, 1)

    x = np.asarray(x, np.float32).astype(np.float16)
    h0 = np.asarray(h0, np.float32).astype(np.float16)
    in_maps = []
    for c in range(NCORES):
        xs = np.ascontiguousarray(
            np.transpose(x[:, c * BL:(c + 1) * BL], (2, 0, 1, 3)))
        h0s = np.ascontiguousarray(
            np.transpose(h0[c * BL:(c + 1) * BL], (1, 0, 2)))
        in_maps.append({
            "x": xs, "h0": h0s,
            "wprz": wprz, "w2rz": w2rz, "wpn": wpn, "w2n": w2n,
            "whhprz": whhprz, "whh1rz": whh1rz,
            "whhpn": whhpn, "whh1n": whh1n,
            "brz": brz, "bihn": bihn, "bhhn": bhhn,
        })
    return in_maps


def kernel(x, h0, w_ih, w_hh, b_ih, b_hh):
    nc = _get_nc()
    in_maps = _prep_in_maps(x, h0, w_ih, w_hh, b_ih, b_hh)
    res = run_bass_kernel_spmd(nc, in_maps, list(range(NCORES)))
    ys = np.empty((T, B, COUT, L), np.float32)
    for c in range(NCORES):
        ys[:, c * BL:(c + 1) * BL] = np.transpose(
            res.results[c]["ys"].astype(np.float32), (0, 2, 1, 3))
    return ys


# revision 24
# speedup vs baseline: 1.0062x; 1.0062x over previous
"""Trainium2 Bass kernel for a convolutional GRU (nn_ConvolutionalRNN).

Reference semantics (per timestep t, torch-GRUCell-style with conv1d gates):
    gi = conv1d(x[t], w_ih) + b_ih          # [B, 3C, L], precomputable
    gh = conv1d(h,    w_hh) + b_hh          # [B, 3C, L], recurrent
    r = sigmoid(gi_r + gh_r); z = sigmoid(gi_z + gh_z)
    n = tanh(gi_n + r * gh_n)
    h = n + z * (h - n)  =  z*h + n*(1-z)
    ys[t] = h

Sharding: data-parallel over batch: B=16 across 8 cores -> BL=2 rows/core.
The two batch rows per core are INDEPENDENT recurrences; they are run as
two skewed software pipelines (chains) so engine work of row 1 fills the
dependency stalls of row 0.

v2 design (vs fp32r baseline at ~950us):
 - fp16 everywhere on chip (PE fp16 = 1 cyc/row always; DVE 16-bit 2x
   modes; x and ys live in HBM as fp16 - host converts, halving DMA).
 - r and z computed in ONE sigmoid over 128 partitions (bias port takes
   the per-partition brz vector).
 - biases folded into free ports: brz -> sigmoid bias, bihn -> tanh bias,
   bhhn -> scalar_tensor_tensor scalar operand. No bias adds anywhere.
 - input-side conv taps 0,1 K-packed to 128 partitions (x is DMA'd twice,
   the second copy shifted by one column) -> 2 MMs instead of 3.
 - per-(t,b) PSUM banks: bankA = pre_rz [128,256], bankB = [gh_n; i_n]
   [128,256] so the skewed chains never share a PSUM bank.
 - z*h runs on GPSIMD (idle engine), zc=1-z on the DVE in the tanh wait
   slot; the critical DVE chain is stt(t1) -> add(t2) -> mul -> add.
 - 32 dummy matmuls at kernel start warm the PE HAM clock gate to 2.4GHz
   (the baseline ran 100% of matmuls at the cold 1.2GHz rate).
"""

import numpy as np
from contextlib import ExitStack

from concourse import bacc, mybir
import concourse.tile as tile
from concourse.bass_utils import run_bass_kernel_spmd

T, B, CIN, COUT, L = 128, 16, 64, 64, 256
GATES = 3 * COUT
NCORES = 8
BL = B // NCORES          # batch rows per core = 2
LP = L + 2                # padded length (zero border at col 0 and L+1)
NB = 6                    # x buffer depth (steps of DMA lookahead)
F32 = mybir.dt.float32
F16 = mybir.dt.float16
AF = mybir.ActivationFunctionType
ALU = mybir.AluOpType


def _build_nc():
    nc = bacc.Bacc(trn_type="TRN2", target_bir_lowering=False, debug=False)

    # Per-core DRAM I/O (fp16 data; fp32 biases).
    x_d = nc.dram_tensor("x", [CIN, T, BL, L], F16, kind="ExternalInput").ap()
    h0_d = nc.dram_tensor("h0", [COUT, BL, L], F16, kind="ExternalInput").ap()
    wprz_d = nc.dram_tensor("wprz", [2 * CIN, 2 * COUT], F16, kind="ExternalInput").ap()
    w2rz_d = nc.dram_tensor("w2rz", [CIN, 2 * COUT], F16, kind="ExternalInput").ap()
    wpn_d = nc.dram_tensor("wpn", [2 * CIN, COUT], F16, kind="ExternalInput").ap()
    w2n_d = nc.dram_tensor("w2n", [CIN, COUT], F16, kind="ExternalInput").ap()
    whhprz_d = nc.dram_tensor("whhprz", [2 * COUT, 2 * COUT], F16, kind="ExternalInput").ap()
    whh1rz_d = nc.dram_tensor("whh1rz", [COUT, 2 * COUT], F16, kind="ExternalInput").ap()
    whhpn_d = nc.dram_tensor("whhpn", [2 * COUT, COUT], F16, kind="ExternalInput").ap()
    whh1n_d = nc.dram_tensor("whh1n", [COUT, COUT], F16, kind="ExternalInput").ap()
    brz_d = nc.dram_tensor("brz", [2 * COUT, 1], F32, kind="ExternalInput").ap()
    bihn_d = nc.dram_tensor("bihn", [COUT, 1], F32, kind="ExternalInput").ap()
    bhhn_d = nc.dram_tensor("bhhn", [COUT, 1], F32, kind="ExternalInput").ap()
    ys_d = nc.dram_tensor("ys", [T, COUT, BL, L], F16, kind="ExternalOutput").ap()

    with tile.TileContext(nc) as tc, ExitStack() as ctx:
        persist = ctx.enter_context(tc.tile_pool(name="persist", bufs=1))
        work = ctx.enter_context(tc.tile_pool(name="work", bufs=2))
        psA = ctx.enter_context(tc.tile_pool(name="psA", bufs=4, space="PSUM"))
        psB = ctx.enter_context(tc.tile_pool(name="psB", bufs=4, space="PSUM"))

        # --- one-time setup -------------------------------------------------
        # Input-side weights, taps 0+1 K-packed (rows 0-63 tap0, 64-127 tap1).
        wprz = persist.tile([2 * CIN, 2 * COUT], F16)
        w2rz = persist.tile([CIN, 2 * COUT], F16)
        wpn = persist.tile([2 * CIN, COUT], F16)
        w2n = persist.tile([CIN, COUT], F16)
        # Recurrent weights, tap-packed like the input side: the h tile keeps
        # h on partitions 64-127 (cols = l+1) and a left-shifted copy on
        # partitions 0-63 (cols = l+1 shifted), so taps (0, 2) run as one
        # K=128 matmul and tap 1 as a K=64 matmul on partitions 64-127.
        whhprz = persist.tile([2 * COUT, 2 * COUT], F16)   # [tap2; tap0]
        whhpn = persist.tile([2 * COUT, COUT], F16)
        whh1rz = persist.tile([2 * COUT, 2 * COUT], F16, name="whh1rz")
        whh1n = persist.tile([2 * COUT, COUT], F16, name="whh1n")
        for t_, d_ in ((wprz, wprz_d), (w2rz, w2rz_d), (wpn, wpn_d),
                       (w2n, w2n_d), (whhprz, whhprz_d), (whhpn, whhpn_d)):
            nc.sync.dma_start(out=t_[:], in_=d_)
        nc.sync.dma_start(out=whh1rz[COUT:2 * COUT], in_=whh1rz_d)
        nc.sync.dma_start(out=whh1n[COUT:2 * COUT], in_=whh1n_d)

        brz = persist.tile([2 * COUT, 1], F32)
        bihn = persist.tile([COUT, 1], F32)
        bhhn = persist.tile([COUT, 1], F32)
        nc.sync.dma_start(out=brz[:], in_=brz_d)
        nc.sync.dma_start(out=bihn[:], in_=bihn_d)
        nc.sync.dma_start(out=bhhn[:], in_=bhhn_d)

        # h state, one tile per batch row, on partitions 64-127; interior
        # cols 1..L, zero halo at cols 0 and L+1.
        hb = [persist.tile([2 * COUT, LP], F16, tag=f"h{b}", name=f"h{b}")
              for b in range(BL)]
        for b in range(BL):
            nc.vector.memset(hb[b][:], 0.0)
            nc.sync.dma_start(out=hb[b][COUT:2 * COUT, 1:L + 1], in_=h0_d[:, b, :])
            nc.sync.dma_start(out=hb[b][0:COUT, 0:L - 1], in_=h0_d[:, b, 1:L])

        # x buffers: rows 0-63 = x shifted right by 1 col (tap0 view),
        # rows 64-127 = x (tap1 view). Col 0 of rows 0-63 must stay zero.
        xbuf = persist.tile([2 * CIN, NB, BL, LP], F16)
        nc.vector.memset(xbuf[:], 0.0)

        def dma_x(t):
            """DMA x for steps t and t+1 in one pair of descriptors (the
            sync-queue DMA-issue instruction costs ~575ns, so batch)."""
            nsteps = min(2, T - t)
            i = t % NB
            xb = xbuf[:, i:i + nsteps]
            nc.sync.dma_start(out=xb[0:CIN, :, :, 1:L + 1],
                              in_=x_d[:, t:t + nsteps])
            nc.sync.dma_start(out=xb[CIN:2 * CIN, :, :, 0:L],
                              in_=x_d[:, t:t + nsteps])

        # --- PE warmup: ~16 dense N=512 matmuls flip the HAM gate ----------
        warm = persist.tile([128, 512], F16)
        nc.vector.memset(warm[:], 0.0)
        wps = psA.tile([128, BL, L], F32, tag="bankA")
        for _ in range(16):
            nc.tensor.matmul(wps[:], warm[:, 0:128], warm[:, 0:2 * L],
                             start=True, stop=True)

        for t in range(0, min(NB - 2, T), 2):
            dma_x(t)

        def wih_mms(t):
            """Input-side convs for step t, both batch rows: 4 MMs at N=512.

            The two skewed chains share these banks; Tile's PSUM bank-hazard
            tracking serializes row 1's recurrent MMs behind row 0's reads,
            which is exactly the skew we want anyway."""
            xb = xbuf[:, t % NB]
            bankA = psA.tile([128, BL, L], F32, tag="bankA")
            bankB = psB.tile([128, BL, L], F32, tag="bankB")
            nc.tensor.matmul(bankA[:], wprz[:], xb[:, :, 0:L],
                             start=True, stop=False)
            nc.tensor.matmul(bankA[:], w2rz[:], xb[0:CIN, :, 2:LP],
                             start=False, stop=False)
            nc.tensor.matmul(bankB[COUT:2 * COUT], wpn[:], xb[:, :, 0:L],
                             start=True, stop=False)
            nc.tensor.matmul(bankB[COUT:2 * COUT], w2n[:], xb[0:CIN, :, 2:LP],
                             start=False, stop=True)
            return bankA, bankB

        def whh_mms(t, b, bankAB):
            """Recurrent convs for (t, b): 4 MMs at N=256, need h(t-1).

            h tile layout: rows 64-127 col c = h[c-1]; rows 0-63 col c =
            h[c+1]. Packed MM at col offset 0 contributes tap0 (h[l-1], rows
            64-127) + tap2 (h[l+1], rows 0-63); the K=64 MM at offset 1
            contributes tap1 (h[l])."""
            bankA, bankB = bankAB
            h = hb[b]
            nc.tensor.matmul(bankA[:, b], whhprz[:], h[:, 0:L],
                             start=False, stop=False)
            nc.tensor.matmul(bankA[:, b], whh1rz[COUT:2 * COUT],
                             h[COUT:2 * COUT, 1:L + 1],
                             start=False, stop=True)
            nc.tensor.matmul(bankB[0:COUT, b], whhpn[:], h[:, 0:L],
                             start=True, stop=False)
            nc.tensor.matmul(bankB[0:COUT, b], whh1n[COUT:2 * COUT],
                             h[COUT:2 * COUT, 1:L + 1],
                             start=False, stop=True)

        # Prime: input convs for step 0.
        banks = [None] * T
        banks[0] = wih_mms(0)

        for t in range(T):
            if t % 2 == 0 and t + NB - 2 < T:
                dma_x(t + NB - 2)
            for b in range(BL):
                bankA, bankB = banks[t]
                whh_mms(t, b, banks[t])
                # Prefetch next step's input convs while this chain's
                # elementwise tail runs (keeps the PE dense and warm).
                if b == 0 and t + 1 < T:
                    banks[t + 1] = wih_mms(t + 1)

                # The whole elementwise tail lives on partitions 64-127 so
                # every SBUF+SBUF operand pair shares a base partition (BIR
                # verifier requirement) and gpsimd work lands on cores 4-7.
                h = hb[b]
                rz = work.tile([2 * COUT, L], F16, tag=f"rz{b}")
                nc.scalar.activation(rz[:], bankA[:, b], AF.Sigmoid,
                                     bias=brz[:])
                # t1 = (gh_n + bhhn) * r ; t2 = t1 + i_n ; n = tanh(t2 + bihn)
                t1 = work.tile([2 * COUT, L], F16, tag=f"t1{b}")
                nc.vector.scalar_tensor_tensor(t1[COUT:2 * COUT],
                                               bankB[0:COUT, b], bhhn[:],
                                               rz[0:COUT], op0=ALU.add,
                                               op1=ALU.mult)
                t2 = work.tile([2 * COUT, L], F16, tag=f"t2{b}")
                nc.vector.tensor_add(t2[COUT:2 * COUT], t1[COUT:2 * COUT],
                                     bankB[COUT:2 * COUT, b])
                n = work.tile([2 * COUT, L], F16, tag=f"n{b}")
                nc.scalar.activation(n[COUT:2 * COUT], t2[COUT:2 * COUT],
                                     AF.Tanh, bias=bihn[:])
                # zc = 1 - z and zh = z*h both on GPSIMD (cores 4-7; all
                # operands on partitions 64-127), keeping the DVE free for
                # the critical chain.
                zc = work.tile([2 * COUT, L], F16, tag=f"zc{b}")
                nc.gpsimd.tensor_scalar(zc[COUT:2 * COUT],
                                        rz[COUT:2 * COUT], -1.0, 1.0,
                                        op0=ALU.mult, op1=ALU.add)
                zh = work.tile([2 * COUT, L], F16, tag=f"zh{b}")
                nc.gpsimd.tensor_mul(zh[COUT:2 * COUT], rz[COUT:2 * COUT],
                                     h[COUT:2 * COUT, 1:L + 1])
                # h_new = z*h + n*zc. The shifted copy (partitions 0-63) is
                # computed first, straight from zh+nzc with shifted column
                # views, so the main h write is the LAST chain op and the
                # shift adds no latency before the next step's matmuls.
                nzc = work.tile([2 * COUT, L], F16, tag=f"nzc{b}")
                nc.vector.tensor_mul(nzc[COUT:2 * COUT], n[COUT:2 * COUT],
                                     zc[COUT:2 * COUT])
                nc.vector.tensor_add(h[0:COUT, 0:L - 1],
                                     zh[COUT:2 * COUT, 1:L],
                                     nzc[COUT:2 * COUT, 1:L])
                nc.vector.tensor_add(h[COUT:2 * COUT, 1:L + 1],
                                     zh[COUT:2 * COUT], nzc[COUT:2 * COUT])
                nc.sync.dma_start(out=ys_d[t, :, b, :],
                                  in_=h[COUT:2 * COUT, 1:L + 1])

    nc.compile()
    return nc


_NC = None


def _get_nc():
    global _NC
    if _NC is None:
        _NC = _build_nc()
    return _NC


def _prep_in_maps(x, h0, w_ih, w_hh, b_ih, b_hh):
    w_ih = np.asarray(w_ih, np.float32)   # [GATES, CIN, 3]
    w_hh = np.asarray(w_hh, np.float32)
    b_ih = np.asarray(b_ih, np.float32)
    b_hh = np.asarray(b_hh, np.float32)

    # lhsT layouts: [K, M] with K = input-channel rows, M = gate cols.
    def packed(w, g0, g1):   # taps 0,1 stacked on K
        return np.concatenate(
            [np.transpose(w[g0:g1, :, 0], (1, 0)),
             np.transpose(w[g0:g1, :, 1], (1, 0))], axis=0).astype(np.float16)

    def tap2(w, g0, g1):
        return np.ascontiguousarray(
            np.transpose(w[g0:g1, :, 2], (1, 0))).astype(np.float16)

    wprz = packed(w_ih, 0, 2 * COUT)
    wpn = packed(w_ih, 2 * COUT, GATES)
    w2rz = tap2(w_ih, 0, 2 * COUT)
    w2n = tap2(w_ih, 2 * COUT, GATES)

    # Recurrent weights: [tap2; tap0] K-packed + tap1 single.
    def hpacked(w, g0, g1):
        return np.concatenate(
            [np.transpose(w[g0:g1, :, 2], (1, 0)),
             np.transpose(w[g0:g1, :, 0], (1, 0))], axis=0).astype(np.float16)

    def htap1(w, g0, g1):
        return np.ascontiguousarray(
            np.transpose(w[g0:g1, :, 1], (1, 0))).astype(np.float16)

    whhprz = hpacked(w_hh, 0, 2 * COUT)
    whh1rz = htap1(w_hh, 0, 2 * COUT)
    whhpn = hpacked(w_hh, 2 * COUT, GATES)
    whh1n = htap1(w_hh, 2 * COUT, GATES)

    brz = (b_ih[:2 * COUT] + b_hh[:2 * COUT]).reshape(2 * COUT, 1)
    bihn = b_ih[2 * COUT:].reshape(COUT, 1)
    bhhn = b_hh[2 * COUT:].reshape(COUT, 1)

    x = np.asarray(x, np.float32).astype(np.float16)
    h0 = np.asarray(h0, np.float32).astype(np.float16)
    in_maps = []
    for c in range(NCORES):
        xs = np.ascontiguousarray(
            np.transpose(x[:, c * BL:(c + 1) * BL], (2, 0, 1, 3)))
        h0s = np.ascontiguousarray(
            np.transpose(h0[c * BL:(c + 1) * BL], (1, 0, 2)))
        in_maps.append({
            "x": xs, "h0": h0s,
            "wprz": wprz, "w2rz": w2rz, "wpn": wpn, "w2n": w2n,
            "whhprz": whhprz, "whh1rz": whh1rz,
            "whhpn": whhpn, "whh1n": whh1n,
            "brz": brz, "bihn": bihn, "bhhn": bhhn,
        })
    return in_maps


def kernel(x, h0, w_ih, w_hh, b_ih, b_hh):
    nc = _get_nc()
    in_maps = _prep_in_maps(x, h0, w_ih, w_hh, b_ih, b_hh)
    res = run_bass_kernel_spmd(nc, in_maps, list(range(NCORES)))
    ys = np.empty((T, B, COUT, L), np.float32)
    for c in range(NCORES):
        ys[:, c * BL:(c + 1) * BL] = np.transpose(
            res.results[c]["ys"].astype(np.float32), (0, 2, 1, 3))
    return ys
